# revision 40
# baseline (speedup 1.0000x reference)
"""MiniGPT (L=8, E=1024, H=16, T=1024, B=4, V=32000) on 8 TRN2 NeuronCores.

Sharding: data-parallel over (batch, sequence-half) -> 8 shards of 512 tokens.
All weights replicated per core (bf16 to enable FWL + halve HBM traffic).
Per layer, the two cores sharing a batch exchange K/V via pair AllGathers
(bf16 payloads). Causal masking is data-driven (per-core mask tables) so the
SPMD program is uniform across cores.

Key optimizations over the f32r baseline:
- bf16 weights + activations on every matmul path (FWL fast-weight-load,
  half the weight DMA), f32 accumulate in PSUM, f32 residual stream.
- LayerNorm affine (g, b) folded into the following weight matrices on the
  host; in-kernel LN is a pure standardize. rstd computed as exp(-0.5*ln(x))
  so Exp/Ln share one ACT table set with the softmax exp.
- Softmax denominators via the ones-column-in-V trick; reciprocal on the DVE
  via reciprocal_approx_fast (single pass) instead of 3.3us iterative divide.
- Scores for two key tiles share a [128,1024] PSUM pair so one ACT exp
  covers 1024 columns (amortizes the 352-cycle ACT ramp).
- fc2 weights pre-arranged host-side so each [128,4096] block is one
  contiguous DMA.
- logits emitted bf16 and upcast on the host.
"""
import sys

sys.path.insert(0, "/opt/trn_rl_repo")

import numpy as np
import ml_dtypes

import concourse.bass as bass
import concourse.bacc as bacc
import concourse.mybir as mybir
import concourse.tile as tile
from concourse.bass_utils import run_bass_kernel_spmd
from concourse.pipe import preload_activation_table

V, E, H, L, T, B = 32000, 1024, 16, 8, 1024, 4
D = E // H              # 64
F = 4 * E               # 4096
EPS = 1e-5
TOK = 512               # tokens per core
NCORES = 8
ET = E // 128            # 8 feature tiles
FT = F // 128            # 32 mlp-hidden tiles
SCALE = 1.0 / np.sqrt(D)

F32 = mybir.dt.float32
F32R = mybir.dt.float32r
BF16 = mybir.dt.bfloat16
AF = mybir.ActivationFunctionType
ALU = mybir.AluOpType

PAIRS = [[0, 1], [2, 3], [4, 5], [6, 7]]
# LM head chunking: 62 chunks of 512 + 1 of 256
HEAD_CHUNKS = [(i * 512, 512) for i in range(62)] + [(62 * 512, 256)]
VSW = H * 65             # V-store width: 16 heads x (64 dims + ones col)

_CACHED = {}
DEBUG = False


def _build_nc():
    debug = DEBUG
    nc = bacc.Bacc("TRN2", target_bir_lowering=False, debug=False,
                   num_devices=NCORES)

    def P(name, shape, dt, out=False):
        return nc.declare_dram_parameter(name, list(shape), dt, isOutput=out)

    x0T = P("x0T", [E, TOK], F32R)                 # per-core residual seed
    wqkvT = P("wqkvT", [L, E, 3 * E], BF16)        # cols: [K | V | Q], g1-folded
    wprojT = P("wprojT", [L, E, E], BF16)
    w1T = P("w1T", [L, E, F], BF16)                # g2-folded
    w2c = P("w2c", [L, ET, 128, F], BF16)          # fc2, contiguous per e-tile
    kqb = P("kqb", [L, 128, 16], F32)              # K bias cols 0-7, Q cols 8-15
    vb = P("vb", [L, 1, E], F32R)                  # V bias row
    b1c = P("b1c", [L, 128, FT], F32)              # fc1 bias as columns
    b2c = P("b2c", [L, 128, ET], F32)              # fc2 bias as columns
    headT = P("headT", [E, V], BF16)               # lnf_g-folded
    masks = P("masks", [4, 128, 2 * TOK], BF16)    # per-core causal masks
    ones_p = P("ones_p", [128, 16], F32R)          # all-ones helper
    logits = P("logits", [TOK, V], BF16, out=True)
    if debug:
        dbg_z1 = P("dbg_z1", [E, TOK], BF16, out=True)
        dbg_kt = P("dbg_kt", [E, 2 * TOK], BF16, out=True)
        dbg_vs = P("dbg_vs", [8, 128, VSW], BF16, out=True)
        dbg_qt = P("dbg_qt", [E, TOK], BF16, out=True)
        dbg_yt = P("dbg_yt", [E, TOK], BF16, out=True)
        dbg_x1 = P("dbg_x1", [E, TOK], F32, out=True)
        dbg_x2 = P("dbg_x2", [E, TOK], F32, out=True)

    from contextlib import ExitStack
    with tile.TileContext(nc) as tc:
        with ExitStack() as _es:
            _p = lambda *a, **k: _es.enter_context(tc.tile_pool(*a, **k))
            persist = _p(name="persist", bufs=1)
            zp = _p(name="zp", bufs=16)          # z/YT [128,512] bf16
            qtp = _p(name="qt", bufs=8)          # QT [128,512] bf16
            utp = _p(name="ut", bufs=FT)         # [128,512] bf16
            wq = _p(name="wq", bufs=18)          # [128,512] bf16 weights
            w2p = _p(name="w2", bufs=2)          # [128,4096] bf16
            stg = _p(name="stg", bufs=4)         # bf16 staging
            stgf = _p(name="stgf", bufs=2)       # f32r LN scratch
            pp = _p(name="pp", bufs=3)           # [128,1024] bf16 probs
            vec = _p(name="vec", bufs=2)
            sm = _p(name="sm", bufs=6)           # [1,512] stats
            rcp = _p(name="rcp", bufs=4)         # [1,512] softmax recip
            pbp = _p(name="pbp", bufs=1)         # [64,512] recip bcast
            psA = _p(name="psA", bufs=2, space="PSUM")   # [128,512]
            psW = _p(name="psW", bufs=2, space="PSUM")   # [128,1024]
            psY = _p(name="psY", bufs=2, space="PSUM")   # [65,512]
            dram = _p(name="dram", bufs=2, space="DRAM")

            # ---- persistent tiles ----
            xT = [persist.tile([128, TOK], F32R, tag=f"xT{e}", name=f"xT{e}")
                  for e in range(ET)]
            KT = [persist.tile([128, 2 * TOK], BF16, tag=f"KT{r}", name=f"KT{r}")
                  for r in range(ET)]
            VS = [persist.tile([128, VSW], BF16, tag=f"VS{t}", name=f"VS{t}")
                  for t in range(8)]
            MK = [persist.tile([128, 2 * TOK], BF16, tag=f"MK{g}", name=f"MK{g}")
                  for g in range(4)]
            ones_col = persist.tile([128, 1], F32R, tag="ones_col")
            ones_row = persist.tile([1, 128], F32R, tag="ones_row")
            eps_t = persist.tile([1, 1], F32, tag="eps")
            act_scr = persist.tile([1, 1], F32, tag="act_scr")
            nc.sync.dma_start(out=ones_col[:], in_=ones_p[:, 0:1])
            nc.sync.dma_start(out=ones_row[:],
                              in_=ones_p.rearrange("a b -> (a b)")[0:128])
            nc.vector.memset(eps_t[:], EPS)

            def warm_mm(dep_ap):
                """Tiny matmul keeping the PE HAM activity window busy while
                the LN scalar/vector chain runs; chained on a chain tile so
                it fires mid-chain. Output is never read."""
                dsc = psA.tile([128, TOK], F32, tag="psA")
                nc.tensor.matmul(dsc[0:64, 0:64], lhsT=ones_row[0:1, 0:64],
                                 rhs=dep_ap, start=True, stop=True)

            for e in range(ET):
                nc.sync.dma_start(out=xT[e][:], in_=x0T[e * 128:(e + 1) * 128, :])
            for g in range(4):
                nc.sync.dma_start(out=MK[g][:], in_=masks[g])

            def layernorm(src):
                """src: ET [128,TOK] f32r tiles. Pure standardize (affine is
                folded into downstream weights). Returns ET bf16 tiles."""
                ps_sum = psY.tile([1, TOK], F32, tag="psY")
                ps_sq = psY.tile([1, TOK], F32, tag="psY")
                for e in range(ET):
                    sq = stgf.tile([128, TOK], F32R, tag="sq")
                    nc.scalar.activation(sq[:], src[e][:], AF.Square)
                    nc.tensor.matmul(ps_sum[:], lhsT=ones_col[:], rhs=src[e][:],
                                     start=(e == 0), stop=(e == ET - 1))
                    nc.tensor.matmul(ps_sq[:], lhsT=ones_col[:], rhs=sq[:],
                                     start=(e == 0), stop=(e == ET - 1))
                mu = sm.tile([1, TOK], F32R, tag="sm")
                e2 = sm.tile([1, TOK], F32R, tag="sm")
                var = sm.tile([1, TOK], F32R, tag="sm")
                nmu = sm.tile([1, TOK], F32R, tag="sm")
                nc.vector.tensor_scalar_mul(mu[:], ps_sum[:], 1.0 / E)
                nc.vector.tensor_scalar_mul(e2[:], ps_sq[:], 1.0 / E)
                warm_mm(mu[0:1, 0:64])
                nc.vector.scalar_tensor_tensor(
                    out=var[:], in0=mu[:], scalar=-1.0, in1=mu[:],
                    op0=ALU.mult, op1=ALU.mult)
                nc.vector.tensor_add(var[:], var[:], e2[:])
                warm_mm(var[0:1, 0:64])
                sd = sm.tile([1, TOK], F32, tag="sm")
                nc.scalar.activation(sd[:], var[:], AF.Sqrt, bias=eps_t[:])
                rsf = sm.tile([1, TOK], F32, tag="sm")
                nc.vector.reciprocal_approx_fast(out=rsf[:], in_=sd[:])
                rstd = sm.tile([1, TOK], F32R, tag="sm")
                with nc.allow_low_precision(reason="f32r rounding for matmul rhs"):
                    nc.vector.tensor_copy(rstd[:], rsf[:])
                rstd_ap = rstd[:]
                nc.vector.scalar_tensor_tensor(
                    out=nmu[:], in0=mu[:], scalar=-1.0, in1=rsf[:],
                    op0=ALU.mult, op1=ALU.mult)
                A = psY.tile([128, TOK], F32, tag="psY")
                C = psY.tile([128, TOK], F32, tag="psY")
                nc.tensor.matmul(A[:], lhsT=ones_row[:], rhs=rstd_ap,
                                 start=True, stop=True)
                nc.tensor.matmul(C[:], lhsT=ones_row[:], rhs=nmu[:],
                                 start=True, stop=True)
                out = []
                for e in range(ET):
                    tmp = stgf.tile([128, TOK], F32R, tag="tmp")
                    nc.vector.tensor_mul(tmp[:], src[e][:], A[:])
                    z = zp.tile([128, TOK], BF16, tag="zp")
                    nc.vector.tensor_add(z[:], tmp[:], C[:])
                    out.append(z)
                return out

            def load_w8(src2d, c0):
                """Load 8 [128,512] bf16 weight tiles covering cols c0:c0+512."""
                wt = [wq.tile([128, 512], BF16, tag="wq", name="wt")
                      for _ in range(ET)]
                for k in range(ET):
                    nc.sync.dma_start(out=wt[k][:],
                                      in_=src2d[k * 128:(k + 1) * 128, c0:c0 + 512])
                return wt

            for l in range(L):
                # stage the Rsqrt ACT table swap while the PE runs prior work
                preload_activation_table(nc.scalar, act_scr, AF.Sqrt)
                # per-layer bias vectors
                kqbt = vec.tile([128, 16], F32, tag="kqbt")
                nc.sync.dma_start(out=kqbt[:], in_=kqb[l])
                vbt = vec.tile([1, E], F32R, tag="vbt")
                nc.sync.dma_start(out=vbt[:], in_=vb[l])
                b1t = vec.tile([128, FT], F32, tag="b1t")
                nc.sync.dma_start(out=b1t[:], in_=b1c[l])
                b2t = vec.tile([128, ET], F32, tag="b2t")
                nc.sync.dma_start(out=b2t[:], in_=b2c[l])

                # ---- LN1 ----
                z1 = layernorm(xT)
                if debug and l == 0:
                    for e in range(ET):
                        nc.sync.dma_start(
                            out=dbg_z1[e * 128:(e + 1) * 128, :], in_=z1[e][:])

                stage_k = [dram.tile([512, TOK], BF16, tag=f"stgk{cb}",
                                     name=f"stage_k{cb}") for cb in range(2)]
                full_k = [dram.tile([2, 512, TOK], BF16, tag=f"fullk{cb}",
                                    name=f"full_k{cb}") for cb in range(2)]
                stage_v = [dram.tile([TOK, 520], BF16, tag=f"stgv{cb}",
                                     name=f"stage_v{cb}") for cb in range(2)]
                full_v = [dram.tile([2, TOK, 520], BF16, tag=f"fullv{cb}",
                                    name=f"full_v{cb}") for cb in range(2)]

                # swap in the Exp table during the K/V/Q Identity copies
                preload_activation_table(nc.scalar, act_scr, AF.Exp)

                # ---- K (wqkvT cols 0..1023), half-gathers so heads 0-7
                # can start before the second half lands ----
                for cb in range(2):
                    wt = load_w8(wqkvT[l], cb * 512)
                    for r in range(4):
                        row = cb * 4 + r
                        pk = psA.tile([128, TOK], F32, tag="psA")
                        for k in range(ET):
                            nc.tensor.matmul(pk[:],
                                             lhsT=wt[k][:, r * 128:(r + 1) * 128],
                                             rhs=z1[k][:], start=(k == 0),
                                             stop=(k == ET - 1))
                        ksb = stg.tile([128, TOK], BF16, tag="stg")
                        nc.scalar.activation(ksb[:], pk[:], AF.Identity,
                                             bias=kqbt[:, row:row + 1])
                        nc.sync.dma_start(
                            out=stage_k[cb][r * 128:(r + 1) * 128, :], in_=ksb[:])
                    nc.gpsimd.collective_compute(
                        "AllGather", ALU.bypass, replica_groups=PAIRS,
                        ins=[stage_k[cb][:]], outs=[full_k[cb][:]])
                    for rk in range(2):
                        for r in range(4):
                            nc.gpsimd.dma_start(
                                out=KT[cb * 4 + r][:, rk * TOK:(rk + 1) * TOK],
                                in_=full_k[cb][rk, r * 128:(r + 1) * 128, :])

                # ---- V (wqkvT cols 1024..2047) ----
                for cb in range(2):
                    wt = load_w8(wqkvT[l], 1024 + cb * 512)
                    for t in range(4):
                        pv = psA.tile([128, 512], F32, tag="psA")
                        nc.tensor.matmul(pv[:], lhsT=ones_row[:],
                                         rhs=vbt[0:1, cb * 512:(cb + 1) * 512],
                                         start=True, stop=False)
                        for k in range(ET):
                            nc.tensor.matmul(pv[:],
                                             lhsT=z1[k][:, t * 128:(t + 1) * 128],
                                             rhs=wt[k][:], start=False,
                                             stop=(k == ET - 1))
                        vsb = stg.tile([128, 520], BF16, tag="stgv")
                        vv = vsb[:].rearrange("p (h d) -> p h d", d=65)
                        nc.vector.memset(vsb[:], 1.0)
                        nc.scalar.activation(vv[:, :, 0:64], pv[:], AF.Identity)
                        nc.sync.dma_start(
                            out=stage_v[cb][t * 128:(t + 1) * 128, :], in_=vsb[:])
                    nc.gpsimd.collective_compute(
                        "AllGather", ALU.bypass, replica_groups=PAIRS,
                        ins=[stage_v[cb][:]], outs=[full_v[cb][:]])
                    for rk in range(2):
                        for t in range(4):
                            nc.gpsimd.dma_start(
                                out=VS[rk * 4 + t][:, cb * 520:(cb + 1) * 520],
                                in_=full_v[cb][rk, t * 128:(t + 1) * 128, :])

                # ---- Q (wqkvT cols 2048..3071) ----
                QT = []
                for cb in range(2):
                    wt = load_w8(wqkvT[l], 2048 + cb * 512)
                    for r in range(4):
                        row = cb * 4 + r
                        pq = psA.tile([128, TOK], F32, tag="psA")
                        for k in range(ET):
                            nc.tensor.matmul(pq[:],
                                             lhsT=wt[k][:, r * 128:(r + 1) * 128],
                                             rhs=z1[k][:], start=(k == 0),
                                             stop=(k == ET - 1))
                        q = qtp.tile([128, TOK], BF16, tag="qt")
                        nc.scalar.activation(q[:], pq[:], AF.Identity,
                                             bias=kqbt[:, 8 + row:9 + row])
                        QT.append(q)

                if debug and l == 0:
                    for r in range(ET):
                        nc.sync.dma_start(out=dbg_kt[r * 128:(r + 1) * 128, :],
                                          in_=KT[r][:])
                        nc.sync.dma_start(out=dbg_qt[r * 128:(r + 1) * 128, :],
                                          in_=QT[r][:])
                    for t8 in range(8):
                        nc.sync.dma_start(out=dbg_vs[t8], in_=VS[t8][:])

                # ---- attention ----
                YT = [zp.tile([128, TOK], BF16, tag="zp", name="yt")
                      for _ in range(ET)]
                for h in range(H):
                    r, po = h // 2, (h % 2) * 64
                    py = psY.tile([65, TOK], F32, tag="psY")
                    probs = []
                    # software pipeline: scores g computed while exp(g-1) runs
                    for g in range(4):
                        pg = psW.tile([128, 2 * TOK], F32, tag="psW")
                        for j in range(2):
                            kt = 2 * g + j
                            nc.tensor.matmul(
                                pg[:, j * TOK:(j + 1) * TOK],
                                lhsT=KT[r][po:po + 64,
                                           kt * 128:(kt + 1) * 128],
                                rhs=QT[r][po:po + 64, :],
                                start=True, stop=True)
                        prob = pp.tile([128, 2 * TOK], BF16, tag="pp")
                        nc.scalar.activation(prob[:], pg[:], AF.Exp,
                                             scale=float(SCALE))
                        nc.vector.tensor_mul(prob[:], prob[:], MK[g][:])
                        probs.append(prob)
                        if g >= 1:  # AV for the previous group
                            pv_ = probs[g - 1]
                            for j in range(2):
                                kt = 2 * (g - 1) + j
                                nc.tensor.matmul(
                                    py[:],
                                    lhsT=VS[kt][:, h * 65:(h + 1) * 65],
                                    rhs=pv_[:, j * TOK:(j + 1) * TOK],
                                    start=(kt == 0), stop=False)
                    for j in range(2):
                        kt = 6 + j
                        nc.tensor.matmul(
                            py[:],
                            lhsT=VS[kt][:, h * 65:(h + 1) * 65],
                            rhs=probs[3][:, j * TOK:(j + 1) * TOK],
                            start=False, stop=(kt == 7))
                    # normalize: row 64 of py is the softmax denominator.
                    # Stage it to SBUF first: the custom-DVE recip reads
                    # garbage from PSUM at a nonzero partition offset.
                    den = rcp.tile([1, TOK], F32, tag="rec")
                    nc.vector.tensor_copy(den[:], py[64:65, :])
                    rec = rcp.tile([1, TOK], F32, tag="rec")
                    nc.vector.reciprocal_approx_fast(out=rec[:], in_=den[:])
                    pbs = pbp.tile([64, TOK], F32, tag="pb")
                    nc.gpsimd.partition_broadcast(pbs[:], rec[:])
                    nc.vector.tensor_mul(YT[r][po:po + 64, :],
                                         py[0:64, :], pbs[:])

                # swap the Rsqrt table back in while the PE runs proj
                preload_activation_table(nc.scalar, act_scr, AF.Sqrt)

                # ---- proj + residual ----
                for cb in range(2):
                    wt = load_w8(wprojT[l], cb * 512)
                    for r in range(4):
                        e = cb * 4 + r
                        pe = psA.tile([128, TOK], F32, tag="psA")
                        for k in range(ET):
                            nc.tensor.matmul(pe[:],
                                             lhsT=wt[k][:, r * 128:(r + 1) * 128],
                                             rhs=YT[k][:], start=(k == 0),
                                             stop=(k == ET - 1))
                        nc.vector.tensor_add(xT[e][:], xT[e][:], pe[:])

                if debug and l == 0:
                    for e in range(ET):
                        nc.sync.dma_start(out=dbg_yt[e * 128:(e + 1) * 128, :],
                                          in_=YT[e][:])
                        nc.sync.dma_start(
                            out=dbg_x1[e * 128:(e + 1) * 128, :],
                            in_=xT[e][:].bitcast(F32))

                # ---- LN2 ----
                z2 = layernorm(xT)

                # swap in the Gelu table while the PE runs the first fc1 chain
                preload_activation_table(nc.scalar, act_scr, AF.Gelu)

                # ---- fc1 + gelu ----
                uT = []
                for cb in range(8):          # 8 chunks of 512 hidden cols
                    wt = load_w8(w1T[l], cb * 512)
                    for r in range(4):
                        uc = cb * 4 + r
                        pu = psA.tile([128, TOK], F32, tag="psA")
                        for k in range(ET):
                            nc.tensor.matmul(pu[:],
                                             lhsT=wt[k][:, r * 128:(r + 1) * 128],
                                             rhs=z2[k][:], start=(k == 0),
                                             stop=(k == ET - 1))
                        u = utp.tile([128, TOK], BF16, tag="ut")
                        nc.scalar.activation(u[:], pu[:], AF.Gelu,
                                             bias=b1t[:, uc:uc + 1])
                        uT.append(u)

                # ---- fc2 + bias + residual ----
                for e in range(ET):
                    w2sb = w2p.tile([128, F], BF16, tag="w2", name="w2sb")
                    nc.sync.dma_start(out=w2sb[:], in_=w2c[l, e])
                    pe = psA.tile([128, TOK], F32, tag="psA")
                    for uc in range(FT):
                        nc.tensor.matmul(
                            pe[:],
                            lhsT=w2sb[:, uc * 128:(uc + 1) * 128],
                            rhs=uT[uc][:], start=(uc == 0), stop=(uc == FT - 1))
                    nc.vector.scalar_tensor_tensor(
                        out=xT[e][:], in0=pe[:], scalar=b2t[:, e:e + 1],
                        in1=xT[e][:], op0=ALU.add, op1=ALU.add)

                if debug and l == 0:
                    for e in range(ET):
                        nc.sync.dma_start(
                            out=dbg_x2[e * 128:(e + 1) * 128, :],
                            in_=xT[e][:].bitcast(F32))

            # ---- final LN + head ----
            preload_activation_table(nc.scalar, act_scr, AF.Sqrt)
            zf = layernorm(xT)

            # head: vocab-chunk pairs share a [128,1024] 2-bank PSUM tile so
            # chains have 4 banks of slack and ACT/DMA counts halve
            for pi in range(31):
                voff = pi * 1024
                wt = [wq.tile([128, 512], BF16, tag="wq", name="wt")
                      for _ in range(2 * ET)]
                for j in range(2):
                    for k in range(ET):
                        nc.sync.dma_start(
                            out=wt[j * ET + k][:],
                            in_=headT[k * 128:(k + 1) * 128,
                                      voff + j * 512:voff + (j + 1) * 512])
                for t in range(4):
                    pw = psW.tile([128, 1024], F32, tag="psW")
                    for j in range(2):
                        for k in range(ET):
                            nc.tensor.matmul(
                                pw[:, j * 512:(j + 1) * 512],
                                lhsT=zf[k][:, t * 128:(t + 1) * 128],
                                rhs=wt[j * ET + k][:], start=(k == 0),
                                stop=(k == ET - 1))
                    lo = stg.tile([128, 1024], BF16, tag="stgl")
                    nc.scalar.activation(lo[:], pw[:], AF.Identity)
                    nc.sync.dma_start(
                        out=logits[t * 128:(t + 1) * 128, voff:voff + 1024],
                        in_=lo[:])
            # tail: vocab 31744..32000
            voff, vlen = 31744, 256
            wt = [wq.tile([128, 512], BF16, tag="wq", name="wt")
                  for _ in range(ET)]
            for k in range(ET):
                nc.sync.dma_start(out=wt[k][:, 0:vlen],
                                  in_=headT[k * 128:(k + 1) * 128,
                                            voff:voff + vlen])
            for t in range(4):
                pl = psA.tile([128, 512], F32, tag="psA")
                for k in range(ET):
                    nc.tensor.matmul(pl[:, 0:vlen],
                                     lhsT=zf[k][:, t * 128:(t + 1) * 128],
                                     rhs=wt[k][:, 0:vlen], start=(k == 0),
                                     stop=(k == ET - 1))
                lo = stg.tile([128, 512], BF16, tag="stg")
                nc.scalar.activation(lo[:, 0:vlen], pl[:, 0:vlen], AF.Identity)
                nc.sync.dma_start(
                    out=logits[t * 128:(t + 1) * 128, voff:voff + vlen],
                    in_=lo[:, 0:vlen])

    nc.finalize()
    return nc


def _host_prep(inputs):
    """Build the 8 per-core input maps from the full model inputs."""
    bf16 = ml_dtypes.bfloat16
    idx = np.asarray(inputs["idx"])
    tok_emb = np.asarray(inputs["tok_emb"], np.float32)
    pos_emb = np.asarray(inputs["pos_emb"], np.float32)
    qkv_w = np.asarray(inputs["qkv_w"], np.float32)
    proj_w = np.asarray(inputs["proj_w"], np.float32)
    fc1_w = np.asarray(inputs["fc1_w"], np.float32)
    fc2_w = np.asarray(inputs["fc2_w"], np.float32)
    head_w = np.asarray(inputs["head_w"], np.float32)
    g1 = np.asarray(inputs["ln1_g"], np.float32)
    b1 = np.asarray(inputs["ln1_b"], np.float32)
    g2 = np.asarray(inputs["ln2_g"], np.float32)
    b2 = np.asarray(inputs["ln2_b"], np.float32)
    gf = np.asarray(inputs["lnf_g"], np.float32)
    bf = np.asarray(inputs["lnf_b"], np.float32)

    # qkv: fold ln1_g into columns, ln1_b into an additive bias
    qkvT = qkv_w.transpose(0, 2, 1) * g1[:, :, None]          # [L, E, 3E]
    wqkvT = np.ascontiguousarray(
        np.concatenate([qkvT[:, :, E:2 * E], qkvT[:, :, 2 * E:3 * E],
                        qkvT[:, :, 0:E]], axis=2)).astype(bf16)  # [K | V | Q]
    qkv_bias = np.einsum('loe,le->lo', qkv_w, b1)             # [L, 3E]
    bias_q = qkv_bias[:, 0:E]
    bias_k = qkv_bias[:, E:2 * E]
    bias_v = qkv_bias[:, 2 * E:3 * E]
    kqb = np.zeros((L, 128, 16), np.float32)
    kqb[:, :, 0:8] = bias_k.reshape(L, 8, 128).transpose(0, 2, 1)
    kqb[:, :, 8:16] = bias_q.reshape(L, 8, 128).transpose(0, 2, 1)
    vb = np.ascontiguousarray(bias_v.reshape(L, 1, E))

    wprojT = np.ascontiguousarray(proj_w.transpose(0, 2, 1)).astype(bf16)

    w1T = np.ascontiguousarray(
        (fc1_w * g2[:, None, :]).transpose(0, 2, 1)).astype(bf16)  # [L, E, F]
    b1eff = np.asarray(inputs["fc1_b"], np.float32) + \
        np.einsum('lfe,le->lf', fc1_w, b2)
    b1c = np.ascontiguousarray(
        b1eff.reshape(L, FT, 128).transpose(0, 2, 1))         # [L,128,FT]

    w2T = fc2_w.transpose(0, 2, 1)                            # [L, F, E]
    w2c = np.ascontiguousarray(
        w2T.reshape(L, FT, 128, ET, 128).transpose(0, 3, 2, 1, 4)
        .reshape(L, ET, 128, F)).astype(bf16)
    b2c = np.ascontiguousarray(
        np.asarray(inputs["fc2_b"], np.float32).reshape(L, ET, 128)
        .transpose(0, 2, 1))                                  # [L,128,ET]

    headTm = np.ascontiguousarray((head_w * gf[None, :]).T).astype(bf16)
    head_host_bias = head_w @ bf                              # [V]

    # causal mask group tiles: group g covers key tiles 2g, 2g+1
    p = np.arange(128)[:, None]
    f = np.arange(TOK)[None, :]
    mj = [(p + 128 * j <= f).astype(np.float32) for j in range(4)]
    zero = np.zeros((128, TOK), np.float32)
    one = np.ones((128, TOK), np.float32)
    m_half0 = np.stack([np.concatenate([mj[0], mj[1]], axis=1),
                        np.concatenate([mj[2], mj[3]], axis=1),
                        np.concatenate([zero, zero], axis=1),
                        np.concatenate([zero, zero], axis=1)]).astype(bf16)
    m_half1 = np.stack([np.concatenate([one, one], axis=1),
                        np.concatenate([one, one], axis=1),
                        np.concatenate([mj[0], mj[1]], axis=1),
                        np.concatenate([mj[2], mj[3]], axis=1)]).astype(bf16)

    x0 = tok_emb[idx] + pos_emb[None, :, :]  # [B, T, E]

    shared = dict(wqkvT=wqkvT, wprojT=wprojT, w1T=w1T, w2c=w2c, kqb=kqb,
                  vb=vb, b1c=b1c, b2c=b2c, headT=headTm,
                  ones_p=np.ones((128, 16), np.float32))
    in_maps = []
    for c in range(NCORES):
        b, half = c // 2, c % 2
        m = dict(shared)
        m["x0T"] = np.ascontiguousarray(
            x0[b, half * TOK:(half + 1) * TOK, :].T).astype(np.float32)
        m["masks"] = np.ascontiguousarray(m_half0 if half == 0 else m_half1)
        in_maps.append(m)
    return in_maps, head_host_bias


LAST_EXEC_NS = None


LAST_RES = None


def kernel(trace=False, trace_cores=None, tmpdir=None, **inputs) -> np.ndarray:
    global LAST_EXEC_NS, LAST_RES
    if "nc" not in _CACHED:
        _CACHED["nc"] = _build_nc()
    nc = _CACHED["nc"]
    in_maps, head_host_bias = _host_prep(inputs)
    res = run_bass_kernel_spmd(nc, in_maps, core_ids=list(range(NCORES)),
                               trace=trace, trace_cores=trace_cores,
                               tmpdir=tmpdir)
    LAST_RES = res
    LAST_EXEC_NS = res.exec_time_ns
    out = np.empty((B, T, V), np.float32)
    for c in range(NCORES):
        b, half = c // 2, c % 2
        out[b, half * TOK:(half + 1) * TOK, :] = \
            res.results[c]["logits"].astype(np.float32)
    if np.any(head_host_bias):
        out += head_host_bias[None, None, :]
    return out


# revision 43
# speedup vs baseline: 1.0275x; 1.0275x over previous
"""MiniGPT (L=8, E=1024, H=16, T=1024, B=4, V=32000) on 8 TRN2 NeuronCores.

Sharding: data-parallel over (batch, sequence-half) -> 8 shards of 512 tokens.
All weights replicated per core (bf16 to enable FWL + halve HBM traffic).
Per layer, the two cores sharing a batch exchange K/V via pair AllGathers
(bf16 payloads). Causal masking is data-driven (per-core mask tables) so the
SPMD program is uniform across cores.

Key optimizations over the f32r baseline:
- bf16 weights + activations on every matmul path (FWL fast-weight-load,
  half the weight DMA), f32 accumulate in PSUM, f32 residual stream.
- LayerNorm affine (g, b) folded into the following weight matrices on the
  host; in-kernel LN is a pure standardize. rstd computed as exp(-0.5*ln(x))
  so Exp/Ln share one ACT table set with the softmax exp.
- Softmax denominators via the ones-column-in-V trick; reciprocal on the DVE
  via reciprocal_approx_fast (single pass) instead of 3.3us iterative divide.
- Scores for two key tiles share a [128,1024] PSUM pair so one ACT exp
  covers 1024 columns (amortizes the 352-cycle ACT ramp).
- fc2 weights pre-arranged host-side so each [128,4096] block is one
  contiguous DMA.
- logits emitted bf16 and upcast on the host.
"""
import sys

sys.path.insert(0, "/opt/trn_rl_repo")

import numpy as np
import ml_dtypes

import concourse.bass as bass
import concourse.bacc as bacc
import concourse.mybir as mybir
import concourse.tile as tile
from concourse.bass_utils import run_bass_kernel_spmd
from concourse.pipe import preload_activation_table

V, E, H, L, T, B = 32000, 1024, 16, 8, 1024, 4
D = E // H              # 64
F = 4 * E               # 4096
EPS = 1e-5
TOK = 512               # tokens per core
NCORES = 8
ET = E // 128            # 8 feature tiles
FT = F // 128            # 32 mlp-hidden tiles
SCALE = 1.0 / np.sqrt(D)

F32 = mybir.dt.float32
F32R = mybir.dt.float32r
BF16 = mybir.dt.bfloat16
AF = mybir.ActivationFunctionType
ALU = mybir.AluOpType

PAIRS = [[0, 1], [2, 3], [4, 5], [6, 7]]
# LM head chunking: 62 chunks of 512 + 1 of 256
HEAD_CHUNKS = [(i * 512, 512) for i in range(62)] + [(62 * 512, 256)]
VSW = H * 65             # V-store width: 16 heads x (64 dims + ones col)

_CACHED = {}
DEBUG = False


def _build_nc():
    debug = DEBUG
    nc = bacc.Bacc("TRN2", target_bir_lowering=False, debug=False,
                   num_devices=NCORES)

    def P(name, shape, dt, out=False):
        return nc.declare_dram_parameter(name, list(shape), dt, isOutput=out)

    x0T = P("x0T", [E, TOK], F32R)                 # per-core residual seed
    wqkvT = P("wqkvT", [L, E, 3 * E], BF16)        # cols: [K | V | Q], g1-folded
    wprojT = P("wprojT", [L, E, E], BF16)
    w1T = P("w1T", [L, E, F], BF16)                # g2-folded
    w2c = P("w2c", [L, ET, 128, F], BF16)          # fc2, contiguous per e-tile
    kqb = P("kqb", [L, 128, 16], F32)              # K bias cols 0-7, Q cols 8-15
    vb = P("vb", [L, 1, E], F32R)                  # V bias row
    b1c = P("b1c", [L, 128, FT], F32)              # fc1 bias as columns
    b2c = P("b2c", [L, 128, ET], F32)              # fc2 bias as columns
    headT = P("headT", [E, V], BF16)               # lnf_g-folded
    masks = P("masks", [4, 128, 2 * TOK], BF16)    # per-core causal masks
    ones_p = P("ones_p", [128, 16], F32R)          # all-ones helper
    logits = P("logits", [TOK, V], BF16, out=True)
    if debug:
        dbg_z1 = P("dbg_z1", [E, TOK], BF16, out=True)
        dbg_kt = P("dbg_kt", [E, 2 * TOK], BF16, out=True)
        dbg_vs = P("dbg_vs", [8, 128, VSW], BF16, out=True)
        dbg_qt = P("dbg_qt", [E, TOK], BF16, out=True)
        dbg_yt = P("dbg_yt", [E, TOK], BF16, out=True)
        dbg_x1 = P("dbg_x1", [E, TOK], F32, out=True)
        dbg_x2 = P("dbg_x2", [E, TOK], F32, out=True)

    from contextlib import ExitStack
    with tile.TileContext(nc) as tc:
        with ExitStack() as _es:
            _p = lambda *a, **k: _es.enter_context(tc.tile_pool(*a, **k))
            persist = _p(name="persist", bufs=1)
            zp = _p(name="zp", bufs=16)          # z/YT [128,512] bf16
            qtp = _p(name="qt", bufs=8)          # QT [128,512] bf16
            utp = _p(name="ut", bufs=FT)         # [128,512] bf16
            wq = _p(name="wq", bufs=18)          # [128,512] bf16 weights
            w2p = _p(name="w2", bufs=2)          # [128,4096] bf16
            stg = _p(name="stg", bufs=4)         # bf16 staging
            stgf = _p(name="stgf", bufs=2)       # f32r LN scratch
            pp = _p(name="pp", bufs=3)           # [128,1024] bf16 probs
            vec = _p(name="vec", bufs=2)
            sm = _p(name="sm", bufs=6)           # [1,512] stats
            rcp = _p(name="rcp", bufs=4)         # [1,512] softmax recip
            pbp = _p(name="pbp", bufs=1)         # [64,512] recip bcast
            psA = _p(name="psA", bufs=2, space="PSUM")   # [128,512]
            psW = _p(name="psW", bufs=2, space="PSUM")   # [128,1024]
            psY = _p(name="psY", bufs=2, space="PSUM")   # [65,512]
            dram = _p(name="dram", bufs=2, space="DRAM")

            # ---- persistent tiles ----
            xT = [persist.tile([128, TOK], F32R, tag=f"xT{e}", name=f"xT{e}")
                  for e in range(ET)]
            KT = [persist.tile([128, 2 * TOK], BF16, tag=f"KT{r}", name=f"KT{r}")
                  for r in range(ET)]
            VS = [persist.tile([128, VSW], BF16, tag=f"VS{t}", name=f"VS{t}")
                  for t in range(8)]
            MK = [persist.tile([128, 2 * TOK], BF16, tag=f"MK{g}", name=f"MK{g}")
                  for g in range(4)]
            ones_col = persist.tile([128, 1], F32R, tag="ones_col")
            ones_row = persist.tile([1, 128], F32R, tag="ones_row")
            eps_t = persist.tile([1, 1], F32, tag="eps")
            act_scr = persist.tile([1, 1], F32, tag="act_scr")
            nc.sync.dma_start(out=ones_col[:], in_=ones_p[:, 0:1])
            nc.sync.dma_start(out=ones_row[:],
                              in_=ones_p.rearrange("a b -> (a b)")[0:128])
            nc.vector.memset(eps_t[:], EPS)

            def warm_mm(dep_ap):
                """Tiny matmul keeping the PE HAM activity window busy while
                the LN scalar/vector chain runs; chained on a chain tile so
                it fires mid-chain. Output is never read."""
                dsc = psA.tile([128, TOK], F32, tag="psA")
                nc.tensor.matmul(dsc[0:64, 0:64], lhsT=ones_row[0:1, 0:64],
                                 rhs=dep_ap, start=True, stop=True)

            for e in range(ET):
                nc.sync.dma_start(out=xT[e][:], in_=x0T[e * 128:(e + 1) * 128, :])
            for g in range(4):
                nc.sync.dma_start(out=MK[g][:], in_=masks[g])

            def layernorm(src):
                """src: ET [128,TOK] f32r tiles. Pure standardize (affine is
                folded into downstream weights). Returns ET bf16 tiles."""
                ps_sum = psY.tile([1, TOK], F32, tag="psY")
                ps_sq = psY.tile([1, TOK], F32, tag="psY")
                for e in range(ET):
                    sq = stgf.tile([128, TOK], F32R, tag="sq")
                    nc.scalar.activation(sq[:], src[e][:], AF.Square)
                    nc.tensor.matmul(ps_sum[:], lhsT=ones_col[:], rhs=src[e][:],
                                     start=(e == 0), stop=(e == ET - 1))
                    nc.tensor.matmul(ps_sq[:], lhsT=ones_col[:], rhs=sq[:],
                                     start=(e == 0), stop=(e == ET - 1))
                mu = sm.tile([1, TOK], F32R, tag="sm")
                e2 = sm.tile([1, TOK], F32R, tag="sm")
                var = sm.tile([1, TOK], F32R, tag="sm")
                nmu = sm.tile([1, TOK], F32R, tag="sm")
                nc.vector.tensor_scalar_mul(mu[:], ps_sum[:], 1.0 / E)
                nc.vector.tensor_scalar_mul(e2[:], ps_sq[:], 1.0 / E)
                warm_mm(mu[0:1, 0:64])
                nc.vector.scalar_tensor_tensor(
                    out=var[:], in0=mu[:], scalar=-1.0, in1=mu[:],
                    op0=ALU.mult, op1=ALU.mult)
                nc.vector.tensor_add(var[:], var[:], e2[:])
                warm_mm(var[0:1, 0:64])
                sd = sm.tile([1, TOK], F32, tag="sm")
                nc.scalar.activation(sd[:], var[:], AF.Sqrt, bias=eps_t[:])
                rsf = sm.tile([1, TOK], F32, tag="sm")
                nc.vector.reciprocal_approx_fast(out=rsf[:], in_=sd[:])
                rstd = sm.tile([1, TOK], F32R, tag="sm")
                with nc.allow_low_precision(reason="f32r rounding for matmul rhs"):
                    nc.vector.tensor_copy(rstd[:], rsf[:])
                rstd_ap = rstd[:]
                warm_mm(rstd[0:1, 0:64])
                nc.vector.scalar_tensor_tensor(
                    out=nmu[:], in0=mu[:], scalar=-1.0, in1=rsf[:],
                    op0=ALU.mult, op1=ALU.mult)
                A = psY.tile([128, TOK], F32, tag="psY")
                C = psY.tile([128, TOK], F32, tag="psY")
                nc.tensor.matmul(A[:], lhsT=ones_row[:], rhs=rstd_ap,
                                 start=True, stop=True)
                nc.tensor.matmul(C[:], lhsT=ones_row[:], rhs=nmu[:],
                                 start=True, stop=True)
                out = []
                for e in range(ET):
                    tmp = stgf.tile([128, TOK], F32R, tag="tmp")
                    nc.vector.tensor_mul(tmp[:], src[e][:], A[:])
                    z = zp.tile([128, TOK], BF16, tag="zp")
                    nc.vector.tensor_add(z[:], tmp[:], C[:])
                    out.append(z)
                return out

            def load_w8(src2d, c0):
                """Load 8 [128,512] bf16 weight tiles covering cols c0:c0+512."""
                wt = [wq.tile([128, 512], BF16, tag="wq", name="wt")
                      for _ in range(ET)]
                for k in range(ET):
                    nc.sync.dma_start(out=wt[k][:],
                                      in_=src2d[k * 128:(k + 1) * 128, c0:c0 + 512])
                return wt

            for l in range(L):
                # per-layer bias vectors
                kqbt = vec.tile([128, 16], F32, tag="kqbt")
                nc.sync.dma_start(out=kqbt[:], in_=kqb[l])
                vbt = vec.tile([1, E], F32R, tag="vbt")
                nc.sync.dma_start(out=vbt[:], in_=vb[l])
                b1t = vec.tile([128, FT], F32, tag="b1t")
                nc.sync.dma_start(out=b1t[:], in_=b1c[l])
                b2t = vec.tile([128, ET], F32, tag="b2t")
                nc.sync.dma_start(out=b2t[:], in_=b2c[l])

                # ---- LN1 ----
                z1 = layernorm(xT)
                if debug and l == 0:
                    for e in range(ET):
                        nc.sync.dma_start(
                            out=dbg_z1[e * 128:(e + 1) * 128, :], in_=z1[e][:])

                stage_k = [dram.tile([512, TOK], BF16, tag=f"stgk{cb}",
                                     name=f"stage_k{cb}") for cb in range(2)]
                full_k = [dram.tile([2, 512, TOK], BF16, tag=f"fullk{cb}",
                                    name=f"full_k{cb}") for cb in range(2)]
                stage_v = [dram.tile([TOK, 520], BF16, tag=f"stgv{cb}",
                                     name=f"stage_v{cb}") for cb in range(2)]
                full_v = [dram.tile([2, TOK, 520], BF16, tag=f"fullv{cb}",
                                    name=f"full_v{cb}") for cb in range(2)]

                # ---- K (wqkvT cols 0..1023), half-gathers so heads 0-7
                # can start before the second half lands ----
                for cb in range(2):
                    wt = load_w8(wqkvT[l], cb * 512)
                    for r in range(4):
                        row = cb * 4 + r
                        pk = psA.tile([128, TOK], F32, tag="psA")
                        for k in range(ET):
                            nc.tensor.matmul(pk[:],
                                             lhsT=wt[k][:, r * 128:(r + 1) * 128],
                                             rhs=z1[k][:], start=(k == 0),
                                             stop=(k == ET - 1))
                        ksb = stg.tile([128, TOK], BF16, tag="stg")
                        nc.scalar.activation(ksb[:], pk[:], AF.Identity,
                                             bias=kqbt[:, row:row + 1])
                        nc.sync.dma_start(
                            out=stage_k[cb][r * 128:(r + 1) * 128, :], in_=ksb[:])
                    nc.gpsimd.collective_compute(
                        "AllGather", ALU.bypass, replica_groups=PAIRS,
                        ins=[stage_k[cb][:]], outs=[full_k[cb][:]])
                    for rk in range(2):
                        for r in range(4):
                            nc.gpsimd.dma_start(
                                out=KT[cb * 4 + r][:, rk * TOK:(rk + 1) * TOK],
                                in_=full_k[cb][rk, r * 128:(r + 1) * 128, :])

                # ---- V (wqkvT cols 1024..2047) ----
                for cb in range(2):
                    wt = load_w8(wqkvT[l], 1024 + cb * 512)
                    for t in range(4):
                        pv = psA.tile([128, 512], F32, tag="psA")
                        nc.tensor.matmul(pv[:], lhsT=ones_row[:],
                                         rhs=vbt[0:1, cb * 512:(cb + 1) * 512],
                                         start=True, stop=False)
                        for k in range(ET):
                            nc.tensor.matmul(pv[:],
                                             lhsT=z1[k][:, t * 128:(t + 1) * 128],
                                             rhs=wt[k][:], start=False,
                                             stop=(k == ET - 1))
                        vsb = stg.tile([128, 520], BF16, tag="stgv")
                        vv = vsb[:].rearrange("p (h d) -> p h d", d=65)
                        nc.vector.memset(vsb[:], 1.0)
                        nc.scalar.activation(vv[:, :, 0:64], pv[:], AF.Identity)
                        nc.sync.dma_start(
                            out=stage_v[cb][t * 128:(t + 1) * 128, :], in_=vsb[:])
                    nc.gpsimd.collective_compute(
                        "AllGather", ALU.bypass, replica_groups=PAIRS,
                        ins=[stage_v[cb][:]], outs=[full_v[cb][:]])
                    for rk in range(2):
                        for t in range(4):
                            nc.gpsimd.dma_start(
                                out=VS[rk * 4 + t][:, cb * 520:(cb + 1) * 520],
                                in_=full_v[cb][rk, t * 128:(t + 1) * 128, :])

                # ---- Q (wqkvT cols 2048..3071) ----
                QT = []
                for cb in range(2):
                    wt = load_w8(wqkvT[l], 2048 + cb * 512)
                    for r in range(4):
                        row = cb * 4 + r
                        pq = psA.tile([128, TOK], F32, tag="psA")
                        for k in range(ET):
                            nc.tensor.matmul(pq[:],
                                             lhsT=wt[k][:, r * 128:(r + 1) * 128],
                                             rhs=z1[k][:], start=(k == 0),
                                             stop=(k == ET - 1))
                        q = qtp.tile([128, TOK], BF16, tag="qt")
                        nc.scalar.activation(q[:], pq[:], AF.Identity,
                                             bias=kqbt[:, 8 + row:9 + row])
                        QT.append(q)

                if debug and l == 0:
                    for r in range(ET):
                        nc.sync.dma_start(out=dbg_kt[r * 128:(r + 1) * 128, :],
                                          in_=KT[r][:])
                        nc.sync.dma_start(out=dbg_qt[r * 128:(r + 1) * 128, :],
                                          in_=QT[r][:])
                    for t8 in range(8):
                        nc.sync.dma_start(out=dbg_vs[t8], in_=VS[t8][:])

                # ---- attention ----
                YT = [zp.tile([128, TOK], BF16, tag="zp", name="yt")
                      for _ in range(ET)]
                for h in range(H):
                    r, po = h // 2, (h % 2) * 64
                    py = psY.tile([65, TOK], F32, tag="psY")
                    probs = []
                    # software pipeline: scores g computed while exp(g-1) runs
                    for g in range(4):
                        pg = psW.tile([128, 2 * TOK], F32, tag="psW")
                        for j in range(2):
                            kt = 2 * g + j
                            nc.tensor.matmul(
                                pg[:, j * TOK:(j + 1) * TOK],
                                lhsT=KT[r][po:po + 64,
                                           kt * 128:(kt + 1) * 128],
                                rhs=QT[r][po:po + 64, :],
                                start=True, stop=True)
                        prob = pp.tile([128, 2 * TOK], BF16, tag="pp")
                        nc.scalar.activation(prob[:], pg[:], AF.Exp,
                                             scale=float(SCALE))
                        nc.vector.tensor_mul(prob[:], prob[:], MK[g][:])
                        probs.append(prob)
                        if g >= 1:  # AV for the previous group
                            pv_ = probs[g - 1]
                            for j in range(2):
                                kt = 2 * (g - 1) + j
                                nc.tensor.matmul(
                                    py[:],
                                    lhsT=VS[kt][:, h * 65:(h + 1) * 65],
                                    rhs=pv_[:, j * TOK:(j + 1) * TOK],
                                    start=(kt == 0), stop=False)
                    for j in range(2):
                        kt = 6 + j
                        nc.tensor.matmul(
                            py[:],
                            lhsT=VS[kt][:, h * 65:(h + 1) * 65],
                            rhs=probs[3][:, j * TOK:(j + 1) * TOK],
                            start=False, stop=(kt == 7))
                    # normalize: row 64 of py is the softmax denominator.
                    # Stage it to SBUF first: the custom-DVE recip reads
                    # garbage from PSUM at a nonzero partition offset.
                    den = rcp.tile([1, TOK], F32, tag="rec")
                    nc.vector.tensor_copy(den[:], py[64:65, :])
                    rec = rcp.tile([1, TOK], F32, tag="rec")
                    nc.vector.reciprocal_approx_fast(out=rec[:], in_=den[:])
                    pbs = pbp.tile([64, TOK], F32, tag="pb")
                    nc.gpsimd.partition_broadcast(pbs[:], rec[:])
                    nc.vector.tensor_mul(YT[r][po:po + 64, :],
                                         py[0:64, :], pbs[:])

                # ---- proj + residual ----
                for cb in range(2):
                    wt = load_w8(wprojT[l], cb * 512)
                    for r in range(4):
                        e = cb * 4 + r
                        pe = psA.tile([128, TOK], F32, tag="psA")
                        for k in range(ET):
                            nc.tensor.matmul(pe[:],
                                             lhsT=wt[k][:, r * 128:(r + 1) * 128],
                                             rhs=YT[k][:], start=(k == 0),
                                             stop=(k == ET - 1))
                        nc.vector.tensor_add(xT[e][:], xT[e][:], pe[:])

                if debug and l == 0:
                    for e in range(ET):
                        nc.sync.dma_start(out=dbg_yt[e * 128:(e + 1) * 128, :],
                                          in_=YT[e][:])
                        nc.sync.dma_start(
                            out=dbg_x1[e * 128:(e + 1) * 128, :],
                            in_=xT[e][:].bitcast(F32))

                # ---- LN2 ----
                z2 = layernorm(xT)

                # ---- fc1 + gelu ----
                uT = []
                for cb in range(8):          # 8 chunks of 512 hidden cols
                    wt = load_w8(w1T[l], cb * 512)
                    for r in range(4):
                        uc = cb * 4 + r
                        pu = psA.tile([128, TOK], F32, tag="psA")
                        for k in range(ET):
                            nc.tensor.matmul(pu[:],
                                             lhsT=wt[k][:, r * 128:(r + 1) * 128],
                                             rhs=z2[k][:], start=(k == 0),
                                             stop=(k == ET - 1))
                        u = utp.tile([128, TOK], BF16, tag="ut")
                        nc.scalar.activation(u[:], pu[:], AF.Gelu,
                                             bias=b1t[:, uc:uc + 1])
                        uT.append(u)

                # ---- fc2 + bias + residual ----
                for e in range(ET):
                    w2sb = w2p.tile([128, F], BF16, tag="w2", name="w2sb")
                    nc.sync.dma_start(out=w2sb[:], in_=w2c[l, e])
                    pe = psA.tile([128, TOK], F32, tag="psA")
                    for uc in range(FT):
                        nc.tensor.matmul(
                            pe[:],
                            lhsT=w2sb[:, uc * 128:(uc + 1) * 128],
                            rhs=uT[uc][:], start=(uc == 0), stop=(uc == FT - 1))
                    nc.vector.scalar_tensor_tensor(
                        out=xT[e][:], in0=pe[:], scalar=b2t[:, e:e + 1],
                        in1=xT[e][:], op0=ALU.add, op1=ALU.add)

                if debug and l == 0:
                    for e in range(ET):
                        nc.sync.dma_start(
                            out=dbg_x2[e * 128:(e + 1) * 128, :],
                            in_=xT[e][:].bitcast(F32))

            # ---- final LN + head ----
            zf = layernorm(xT)

            # head: one 512-vocab chunk per chain, rotating over 4 PSUM
            # banks (psA x2 + psY x2) so chains never wait on an ACT drain
            for ci, (voff, vlen) in enumerate(HEAD_CHUNKS):
                wt = [wq.tile([128, 512], BF16, tag="wq", name="wt")
                      for _ in range(ET)]
                for k in range(ET):
                    nc.sync.dma_start(out=wt[k][:, 0:vlen],
                                      in_=headT[k * 128:(k + 1) * 128,
                                                voff:voff + vlen])
                for t in range(4):
                    if t % 2 == 0:
                        pl = psA.tile([128, 512], F32, tag="psA")
                    else:
                        pl = psY.tile([128, 512], F32, tag="psY")
                    for k in range(ET):
                        nc.tensor.matmul(pl[:, 0:vlen],
                                         lhsT=zf[k][:, t * 128:(t + 1) * 128],
                                         rhs=wt[k][:, 0:vlen], start=(k == 0),
                                         stop=(k == ET - 1))
                    lo = stg.tile([128, 512], BF16, tag="stg")
                    nc.scalar.activation(lo[:, 0:vlen], pl[:, 0:vlen],
                                         AF.Identity)
                    nc.sync.dma_start(
                        out=logits[t * 128:(t + 1) * 128, voff:voff + vlen],
                        in_=lo[:, 0:vlen])

    nc.finalize()
    return nc


def _host_prep(inputs):
    """Build the 8 per-core input maps from the full model inputs."""
    bf16 = ml_dtypes.bfloat16
    idx = np.asarray(inputs["idx"])
    tok_emb = np.asarray(inputs["tok_emb"], np.float32)
    pos_emb = np.asarray(inputs["pos_emb"], np.float32)
    qkv_w = np.asarray(inputs["qkv_w"], np.float32)
    proj_w = np.asarray(inputs["proj_w"], np.float32)
    fc1_w = np.asarray(inputs["fc1_w"], np.float32)
    fc2_w = np.asarray(inputs["fc2_w"], np.float32)
    head_w = np.asarray(inputs["head_w"], np.float32)
    g1 = np.asarray(inputs["ln1_g"], np.float32)
    b1 = np.asarray(inputs["ln1_b"], np.float32)
    g2 = np.asarray(inputs["ln2_g"], np.float32)
    b2 = np.asarray(inputs["ln2_b"], np.float32)
    gf = np.asarray(inputs["lnf_g"], np.float32)
    bf = np.asarray(inputs["lnf_b"], np.float32)

    # qkv: fold ln1_g into columns, ln1_b into an additive bias
    qkvT = qkv_w.transpose(0, 2, 1) * g1[:, :, None]          # [L, E, 3E]
    wqkvT = np.ascontiguousarray(
        np.concatenate([qkvT[:, :, E:2 * E], qkvT[:, :, 2 * E:3 * E],
                        qkvT[:, :, 0:E]], axis=2)).astype(bf16)  # [K | V | Q]
    qkv_bias = np.einsum('loe,le->lo', qkv_w, b1)             # [L, 3E]
    bias_q = qkv_bias[:, 0:E]
    bias_k = qkv_bias[:, E:2 * E]
    bias_v = qkv_bias[:, 2 * E:3 * E]
    kqb = np.zeros((L, 128, 16), np.float32)
    kqb[:, :, 0:8] = bias_k.reshape(L, 8, 128).transpose(0, 2, 1)
    kqb[:, :, 8:16] = bias_q.reshape(L, 8, 128).transpose(0, 2, 1)
    vb = np.ascontiguousarray(bias_v.reshape(L, 1, E))

    wprojT = np.ascontiguousarray(proj_w.transpose(0, 2, 1)).astype(bf16)

    w1T = np.ascontiguousarray(
        (fc1_w * g2[:, None, :]).transpose(0, 2, 1)).astype(bf16)  # [L, E, F]
    b1eff = np.asarray(inputs["fc1_b"], np.float32) + \
        np.einsum('lfe,le->lf', fc1_w, b2)
    b1c = np.ascontiguousarray(
        b1eff.reshape(L, FT, 128).transpose(0, 2, 1))         # [L,128,FT]

    w2T = fc2_w.transpose(0, 2, 1)                            # [L, F, E]
    w2c = np.ascontiguousarray(
        w2T.reshape(L, FT, 128, ET, 128).transpose(0, 3, 2, 1, 4)
        .reshape(L, ET, 128, F)).astype(bf16)
    b2c = np.ascontiguousarray(
        np.asarray(inputs["fc2_b"], np.float32).reshape(L, ET, 128)
        .transpose(0, 2, 1))                                  # [L,128,ET]

    headTm = np.ascontiguousarray((head_w * gf[None, :]).T).astype(bf16)
    head_host_bias = head_w @ bf                              # [V]

    # causal mask group tiles: group g covers key tiles 2g, 2g+1
    p = np.arange(128)[:, None]
    f = np.arange(TOK)[None, :]
    mj = [(p + 128 * j <= f).astype(np.float32) for j in range(4)]
    zero = np.zeros((128, TOK), np.float32)
    one = np.ones((128, TOK), np.float32)
    m_half0 = np.stack([np.concatenate([mj[0], mj[1]], axis=1),
                        np.concatenate([mj[2], mj[3]], axis=1),
                        np.concatenate([zero, zero], axis=1),
                        np.concatenate([zero, zero], axis=1)]).astype(bf16)
    m_half1 = np.stack([np.concatenate([one, one], axis=1),
                        np.concatenate([one, one], axis=1),
                        np.concatenate([mj[0], mj[1]], axis=1),
                        np.concatenate([mj[2], mj[3]], axis=1)]).astype(bf16)

    x0 = tok_emb[idx] + pos_emb[None, :, :]  # [B, T, E]

    shared = dict(wqkvT=wqkvT, wprojT=wprojT, w1T=w1T, w2c=w2c, kqb=kqb,
                  vb=vb, b1c=b1c, b2c=b2c, headT=headTm,
                  ones_p=np.ones((128, 16), np.float32))
    in_maps = []
    for c in range(NCORES):
        b, half = c // 2, c % 2
        m = dict(shared)
        m["x0T"] = np.ascontiguousarray(
            x0[b, half * TOK:(half + 1) * TOK, :].T).astype(np.float32)
        m["masks"] = np.ascontiguousarray(m_half0 if half == 0 else m_half1)
        in_maps.append(m)
    return in_maps, head_host_bias


LAST_EXEC_NS = None


LAST_RES = None


def kernel(trace=False, trace_cores=None, tmpdir=None, **inputs) -> np.ndarray:
    global LAST_EXEC_NS, LAST_RES
    if "nc" not in _CACHED:
        _CACHED["nc"] = _build_nc()
    nc = _CACHED["nc"]
    in_maps, head_host_bias = _host_prep(inputs)
    res = run_bass_kernel_spmd(nc, in_maps, core_ids=list(range(NCORES)),
                               trace=trace, trace_cores=trace_cores,
                               tmpdir=tmpdir)
    LAST_RES = res
    LAST_EXEC_NS = res.exec_time_ns
    out = np.empty((B, T, V), np.float32)
    for c in range(NCORES):
        b, half = c // 2, c % 2
        out[b, half * TOK:(half + 1) * TOK, :] = \
            res.results[c]["logits"].astype(np.float32)
    if np.any(head_host_bias):
        out += head_host_bias[None, None, :]
    return out
